# revision 60
# baseline (speedup 1.0000x reference)
"""Bamba attention decoder layer on 8 Trainium2 NeuronCores.

Sharding: tensor-parallel attention (4 q heads + 1 kv head per core),
AllToAll of attention context, token-sliced o_proj + fused add, chunked
AllGather of the *unnormalized* residual (the 1/rms factor commutes through
the gate/up matmuls and is applied on the consumer side), I-sharded SwiGLU
MLP (1792 cols/core) fused with the down projection per token half (h stays
in SBUF), ReduceScatter of down-proj partials.

All four projections (qkv, o_proj, gate_up, down) run as split-precision
fp8e4 DoubleRow matmuls: each operand X is represented as X_hi + X_lo (both
e4m3), and the product keeps the three dominant cross terms
Wh*Xh + Wh*Xl + Wl*Xh (the dropped Wl*Xl term is ~0.07%).  A DoubleRow
matmul contracts two 128-deep k-tiles per instruction at 0.5 cycles/row, so
the three terms per 256-K pair cost 0.75x the bf16 cycles while keeping
~bf16-level accuracy.  Weights are pre-scaled by 2^6 on the host (their
sigma=0.02 values would otherwise land in the fp8 subnormal range); the
2^-6 descale folds into the rope tables, the rstd rows, the fused
residual-add and the down-proj psum copy.  Attention stays bf16 (exp values
overflow fp8).  The residual stream, rmsnorm statistics and kernel outputs
stay fp32/bf16.
"""

import numpy as np
import ml_dtypes

import concourse.bacc as bacc
import concourse.bass_isa as bass_isa
import concourse.mybir as mybir
import concourse.tile as tile
from concourse.bass_utils import run_bass_kernel_spmd
from concourse.masks import make_identity

NC = 8
S = 2048
H = 4096
HD = 128
NQ = 32
NKV = 8
I = 14336
QH = NQ // NC        # q heads per core = 4
IPC = I // NC        # intermediate cols per core = 1792
TPC = S // NC        # tokens per core = 256
EPS = 1e-5
THETA = 10000.0
SCALE = HD ** -0.5
SW = 2.0 ** 6        # host-side weight scale (fp8 subnormal avoidance)
DSW = 2.0 ** -6

F32 = mybir.dt.float32
F32R = mybir.dt.float32r
BF16 = mybir.dt.bfloat16
FP8 = mybir.dt.float8e4
E4NP = ml_dtypes.float8_e4m3

KH = H // 128        # 32 k-tiles over H
NB = S // 512        # 4 token blocks of 512
MB_GU = IPC // 128   # 14 m tiles for gate (and for up)
KI = IPC // 128      # 14 k tiles over I per core
SH = S // 2          # tokens per half = 1024
# AllGather chunking of the residual stream: smaller tail chunks so the
# last chunk's transfer chain at the o_proj->MLP boundary is short
AG_CHUNKS = [(0, 8), (8, 8), (16, 6), (22, 4), (26, 4), (30, 2)]

AF = mybir.ActivationFunctionType
DR = mybir.MatmulPerfMode.DoubleRow
ALU = mybir.AluOpType


def _dr3(nc, out_ps, wh, wl, xh, xl, nkt, wk0=0, ncols=None, first=False,
         last=False, wmid=None, xmid=None, skip_gc=False):
    """Accumulate the 3-term split-fp8 product over k-tile pairs into
    out_ps: Wh*Xh + Wh*Xl + Wl*Xh per pair (the Wl*Xl term is dropped).
    xh/xl are indexed 0..nkt; the weight k-tiles start at wk0.  nkt must be
    even.  ncols optionally slices the moving columns.  wmid/xmid index an
    extra leading dim for 4D [128, mid, kt, c] tiles."""
    terms = []
    for t in range(0, nkt, 2):
        wsl = slice(wk0 + t, wk0 + t + 2)
        xsl = slice(t, t + 2)
        terms.append((wh, xh, wsl, xsl))
        terms.append((wh, xl, wsl, xsl))
        terms.append((wl, xh, wsl, xsl))
    n = len(terms)
    for i, (a, b, wsl, xsl) in enumerate(terms):
        lhsT = a[:, wsl, :] if wmid is None else a[:, wmid, wsl, :]
        csl = slice(None) if ncols is None else ncols
        rhs = b[:, xsl, csl] if xmid is None else b[:, xmid, xsl, csl]
        nc.tensor.matmul(
            out_ps[:], lhsT, rhs,
            start=(first and i == 0), stop=(last and i == n - 1),
            perf_mode=DR, skip_group_check=skip_gc,
        )


def _attn_block(nc, g, p2s, p2p, hh, qb, with_collectives):
    """Causal GQA attention for head hh, q-block qb (512 q tokens)."""
    nkt = 4 * qb + 4
    att_ps = p2p.tile([128, 512], F32, name="att_ps", tag="att_ps", bufs=1)
    sums_ps = p2p.tile([1, 512], F32, name="sums_ps", tag="sums_ps", bufs=1)
    # waves: issue score matmuls back-to-back, then their att/sums
    # accumulations — by the time the PE (in-order) reaches an att matmul,
    # its exp+mask chain has drained, so it doesn't bubble per tile
    for w0 in range(0, nkt, 16):
        wave = range(w0, min(w0 + 16, nkt))
        es = []
        for kt in wave:
            j = kt - 4 * qb
            # on diagonal tiles only q-columns >= 128*j attend at all;
            # skip the fully-masked column range entirely
            q0 = 128 * j if j > 0 else 0
            lsl = slice(q0, 512)
            s_ps = p2p.tile([128, 512], F32, name="s_ps", tag="s_ps", bufs=4)
            nc.tensor.matmul(
                s_ps[:, lsl], g["kT_sb"][:, kt * 128:(kt + 1) * 128],
                g["qT_sb"][:, hh, qb * 512 + q0:(qb + 1) * 512],
                start=True, stop=True,
            )
            e = p2s.tile([128, 512], BF16, name="e", tag="e", bufs=8)
            nc.scalar.activation(e[:, lsl], s_ps[:, lsl], AF.Exp, scale=SCALE)
            if j >= 0:
                nc.vector.tensor_mul(e[:, lsl], e[:, lsl], g["mask_sb"][:, j, lsl])
            es.append((kt, lsl, e))
        for kt, lsl, e in es:
            nc.tensor.matmul(att_ps[:, lsl], g["v_tok"][:, kt, :], e[:, lsl],
                             start=(kt == 0), stop=(kt == nkt - 1),
                             skip_group_check=True)
            nc.tensor.matmul(sums_ps[:, lsl], g["ones_b"][:], e[:, lsl],
                             start=(kt == 0), stop=(kt == nkt - 1),
                             skip_group_check=True)
    recip = p2s.tile([1, 512], F32, name="recip", tag="recip")
    nc.vector.reciprocal(recip[:], sums_ps[:])
    rb2 = p2s.tile([128, 512], F32, name="rb2", tag="rb2", bufs=1)
    nc.gpsimd.partition_broadcast(rb2[:], recip[:])
    # split the normalized context into fp8 hi+lo straight into the
    # SBUF-resident context tiles the row-parallel o_proj contracts over
    an32 = p2s.tile([128, 512], F32R, name="an32", tag="an32", bufs=1)
    nc.vector.tensor_mul(an32[:], att_ps[:], rb2[:])
    csl = slice(qb * 512, (qb + 1) * 512)
    nc.vector.tensor_copy(g["ctxh_sb"][:, hh, csl], an32[:])
    nc.vector.tensor_sub(g["ctxl_sb"][:, hh, csl], an32[:],
                         g["ctxh_sb"][:, hh, csl])


def _phase12_qkv_attn(nc, tc, g, p2s, with_collectives, rg):
    p1p = g["psum"]
    """QKV matmul + rope, fused with attention: the attention for token
    block nb runs right after block nb's rope, filling the PE while the
    exp/softmax pipeline of earlier blocks drains.  The rmsnorm1 factor is
    precomputed on the host (it only depends on the kernel input) and folded
    into cosT/sinT; v is scaled by the hosted rstd1 row directly."""
    with (
        tc.tile_pool(name="p1sbuf", bufs=2) as p1s,
        tc.tile_pool(name="p1w", bufs=1) as p1w,
    ):
        # first-needed-first DMA order: block-0 activations + first weight
        # chunk go ahead of everything else so the PE starts within ~5us.
        def load_block(nb):
            ncols = slice(nb * 512, (nb + 1) * 512)
            hhs, hls = [], []
            for kc in range(4):
                hh_ = p1s.tile([128, 8, 512], FP8, name="hbh", tag="hbh", bufs=6)
                nc.sync.dma_start(hh_[:], g["hTbh"][:, kc * 8:(kc + 1) * 8, ncols])
                hhs.append(hh_)
                hl_ = p1s.tile([128, 8, 512], FP8, name="hbl", tag="hbl", bufs=6)
                nc.sync.dma_start(hl_[:], g["hTbl"][:, kc * 8:(kc + 1) * 8, ncols])
                hls.append(hl_)
            return hhs, hls

        hb_h, hb_l = [], []
        for kc in range(4):
            hb_h.append(p1s.tile([128, 8, 512], FP8, name="hbh", tag="hbh", bufs=6))
            hb_l.append(p1s.tile([128, 8, 512], FP8, name="hbl", tag="hbl", bufs=6))
        wq_h, wq_l = [], []
        for m in range(QH + 2):
            wq_h.append(p1w.tile([128, KH, 128], FP8, name=f"wqh{m}"))
            wq_l.append(p1w.tile([128, KH, 128], FP8, name=f"wql{m}"))
        # m-major weight chunks interleaved with block-0 activations: m0's
        # first weights + the first activation chunk arrive within ~2us
        nc.sync.dma_start(hb_h[0][:, 0:4, :], g["hTbh"][:, 0:4, 0:512])
        nc.sync.dma_start(wq_h[0][:], g["wqkvh"][:, 0, :, :])
        nc.sync.dma_start(hb_l[0][:, 0:4, :], g["hTbl"][:, 0:4, 0:512])
        nc.sync.dma_start(wq_l[0][:], g["wqkvl"][:, 0, :, :])
        nc.sync.dma_start(hb_h[0][:, 4:8, :], g["hTbh"][:, 4:8, 0:512])
        nc.sync.dma_start(hb_l[0][:, 4:8, :], g["hTbl"][:, 4:8, 0:512])
        nc.sync.dma_start(wq_h[1][:], g["wqkvh"][:, 1, :, :])
        nc.sync.dma_start(wq_l[1][:], g["wqkvl"][:, 1, :, :])
        for kc in range(1, 4):
            nc.sync.dma_start(hb_h[kc][:], g["hTbh"][:, kc * 8:(kc + 1) * 8, 0:512])
            nc.sync.dma_start(hb_l[kc][:], g["hTbl"][:, kc * 8:(kc + 1) * 8, 0:512])
        cos_sb = p1w.tile([128, S], BF16, name="cos_sb")
        nc.sync.dma_start(cos_sb[:, 0:512], g["cosT"][:, 0:512])
        sin_sb = p1w.tile([128, S], BF16, name="sin_sb")
        nc.sync.dma_start(sin_sb[:, 0:512], g["sinT"][:, 0:512])
        rstd1 = g["rstd1_sb"]
        nc.sync.dma_start(rstd1[:], g["rstd1"][:, :])
        for m in range(2, QH + 2):
            nc.sync.dma_start(wq_h[m][:], g["wqkvh"][:, m, :, :])
            nc.sync.dma_start(wq_l[m][:], g["wqkvl"][:, m, :, :])
        nc.sync.dma_start(cos_sb[:, 512:S], g["cosT"][:, 512:S])
        nc.sync.dma_start(sin_sb[:, 512:S], g["sinT"][:, 512:S])
        nc.sync.dma_start(g["mask_sb"][:], g["masks"][:, :, :])

        nxt = load_block(1)   # double-buffered block prefetch
        for nb in range(NB):
            ncols = slice(nb * 512, (nb + 1) * 512)
            if nb == 0:
                hhs, hls = hb_h, hb_l
            else:
                hhs, hls = nxt
                nxt = load_block(nb + 1) if nb + 1 < NB else None
            if nb == 2:
                # hT_slice for the o_proj residual add: load it during the
                # DMA-quiet attention stretch, not o_proj's saturated window
                nc.sync.dma_start(g["hsl_sb"][:], g["hT_slice"][:, :, :])
            rb = p1s.tile([128, 512], F32, name="rb", tag="rb", bufs=2)
            nc.gpsimd.partition_broadcast(rb[:], rstd1[:, ncols])

            def finish_m(m, mm):
                if m < QH + 1:
                    qkc = p1s.tile([128, 512], F32, name="qkc", tag="qkc", bufs=1)
                    nc.scalar.copy(qkc[:], mm[:])
                    if m < QH:
                        d0 = g["qT_sb"][0:64, m, ncols]
                        d1 = g["qT_sb"][64:128, m, ncols]
                    else:
                        d0 = g["kT_sb"][0:64, ncols]
                        d1 = g["kT_sb"][64:128, ncols]
                    t0 = p1s.tile([64, 512], F32, name="t0", tag="t0", bufs=1)
                    nc.vector.tensor_mul(t0[:], qkc[0:64, :], cos_sb[0:64, ncols])
                    t1 = p1s.tile([64, 512], F32, name="t1", tag="t1", bufs=1)
                    nc.vector.tensor_mul(t1[:], qkc[64:128, :], sin_sb[64:128, ncols])
                    nc.vector.tensor_sub(d0, t0[:], t1[:])
                    t2 = p1s.tile([64, 512], F32, name="t2", tag="t0", bufs=1)
                    nc.vector.tensor_mul(t2[:], qkc[64:128, :], cos_sb[64:128, ncols])
                    t3 = p1s.tile([64, 512], F32, name="t3", tag="t1", bufs=1)
                    nc.vector.tensor_mul(t3[:], qkc[0:64, :], sin_sb[0:64, ncols])
                    nc.vector.tensor_add(d1, t2[:], t3[:])
                else:
                    vtmp = p1s.tile([128, 512], BF16, name="vtmp", tag="vtmp", bufs=1)
                    nc.vector.tensor_mul(vtmp[:], mm[:], rb[:])
                    for j in range(4):
                        # shares the attention-score psum tag (bank budget)
                        tp = p1p.tile([128, 128], BF16, name="tp", tag="s_ps",
                                      bufs=4)
                        nc.tensor.transpose(tp[:], vtmp[:, j * 128:(j + 1) * 128],
                                            g["ident"][:])
                        nc.vector.tensor_copy(g["v_tok"][:, nb * 4 + j, :], tp[:])

            # m-outer: one accumulating psum at a time (block-resident hb);
            # the previous block's attention interleaves between this
            # block's qkv chains, so each ACT-paced attention unit is
            # followed by a dense matmul chain and the in-order PE doesn't
            # idle on the exp pipeline
            if nb == 0:
                # kc-outer for the first block: the PE starts on activation
                # chunk 0 while chunks 1-3 are still streaming in (6 psum
                # groups stay open, one per output m-tile)
                # reuse the attention-phase psum tags (idle during block 0)
                mm_tags = [("s_ps", 4), ("s_ps", 4), ("s_ps", 4), ("s_ps", 4),
                           ("att_ps", 1), ("mm", 2)]
                mms = [p1p.tile([128, 512], F32, name=f"mm0_{m}", tag=tg,
                                bufs=bf) for m, (tg, bf) in enumerate(mm_tags)]
                for kc in range(4):
                    for m in range(QH + 2):
                        _dr3(nc, mms[m], wq_h[m], wq_l[m], hhs[kc], hls[kc], 8,
                             wk0=kc * 8, first=(kc == 0), last=(kc == 3),
                             skip_gc=True)
                for m in range(QH + 2):
                    finish_m(m, mms[m])
            else:
                for m in range(QH + 2):
                    mm = p1p.tile([128, 512], F32, name="mm", tag="mm", bufs=2)
                    for kc in range(4):
                        # hb tiles hold k-tiles kc*8..kc*8+8 at dim-1 index 0..8
                        _dr3(nc, mm, wq_h[m], wq_l[m], hhs[kc], hls[kc], 8,
                             wk0=kc * 8, first=(kc == 0), last=(kc == 3))
                    finish_m(m, mm)
                    if m < QH:
                        _attn_block(nc, g, p2s, p1p, m, nb - 1, with_collectives)
        # the last block's attention runs after its rope; its ctx writes
        # ship each head's context as it completes
        for hh in range(QH):
            _attn_block(nc, g, p2s, p1p, hh, NB - 1, with_collectives)


def _phase3_oproj(nc, tc, g, with_collectives, rg):
    """Row-parallel o_proj: each core contracts only its own 4 heads
    (K=512) against its w_o row slice and ReduceScatters bf16 partials
    (chunked by output k-tile range), receiving its 256-token slice summed.
    Then the fused descale + residual add, sum-of-squares, and the chunked
    AllGather of the unnormalized residual (fp8 hi+lo).  First-token-half
    xn loads ride the Pool DMA queue as gathered chunks land."""
    with (
        tc.tile_pool(name="p3sbuf", bufs=2) as p3s,
        tc.tile_pool(name="p3big", bufs=1) as p3b,
    ):
        p3p = g["psum"]
        # o_proj weights (own-head row slice, 4.2 MB)
        wosh = p3b.tile([128, KH, QH, 128], FP8, name="wosh")
        nc.sync.dma_start(wosh[:], g["woh"][:, :, :, :])
        wosl = p3b.tile([128, KH, QH, 128], FP8, name="wosl")
        nc.sync.dma_start(wosl[:], g["wol"][:, :, :, :])
        ctxh, ctxl = g["ctxh_sb"], g["ctxl_sb"]   # written during attention
        hsl = g["hsl_sb"]   # preloaded during phase 1
        st2_ps = p3p.tile([1, TPC], F32, name="st2_ps", tag="sums_ps", bufs=1)
        rsbs = []

        def rsb_load(cch, off, sz):
            rsb = p3s.tile([128, sz, TPC], BF16, name="rsb", tag="rsb", bufs=2)
            nc.sync.dma_start(rsb[:], g[f"rsp_out{cch}"][:, :, :])
            return rsb

        def consumer(cch, off, sz, rsb):
            """RS-result consumption for one chunk: fused residual add,
            sum-of-squares, bf16 + fp8 hi/lo copies, AllGather, xn loads.
            Deferred one chunk behind the partial matmuls so the RS round
            trip never stalls the in-order PE queue."""
            x2b = p3b.tile([128, sz, TPC], BF16, name=f"x2b{cch}")
            x2h = p3b.tile([128, sz, TPC], FP8, name=f"x2h{cch}")
            x2l = p3b.tile([128, sz, TPC], FP8, name=f"x2l{cch}")
            g[f"x2b{cch}"] = x2b
            for mi in range(sz):
                m = off + mi
                # fused descale + residual add: res2 = rs * 2^-6 + hsl
                res2 = p3s.tile([128, TPC], F32, name="res2", tag="res2", bufs=2)
                nc.vector.scalar_tensor_tensor(
                    res2[:], rsb[:, mi, :], DSW, hsl[:, m, :], ALU.mult, ALU.add)
                sq2 = p3s.tile([128, TPC], F32R, name="sq2", tag="sq2", bufs=2)
                nc.scalar.activation(sq2[:], res2[:], AF.Square)
                nc.tensor.matmul(st2_ps[:], g["ones"][:], sq2[:],
                                 start=(m == 0), stop=(m == KH - 1),
                                 skip_group_check=True)
                nc.vector.tensor_copy(x2b[:, mi, :], res2[:])
                nc.vector.tensor_copy(x2h[:, mi, :], res2[:])
                nc.vector.tensor_sub(x2l[:, mi, :], res2[:], x2h[:, mi, :])
                if m == 0:
                    # prefetch the first gate/up weights off the Pool
                    # queue so they're resident when the MLP starts
                    nc.gpsimd.dma_start(g["gb0h"][:], g["wguh"][:, 0, :, :])
                    nc.gpsimd.dma_start(g["gb0l"][:], g["wgul"][:, 0, :, :])
                    nc.gpsimd.dma_start(g["ub0h"][:], g["wguh"][:, MB_GU, :, :])
                    nc.gpsimd.dma_start(g["ub0l"][:], g["wgul"][:, MB_GU, :, :])
            # AllGather chain first (it feeds the MLP's critical path),
            # res_out write after.  Fallback writes the core's own slice
            # straight to ag2_out, skipping the staging copy.
            if with_collectives:
                nc.sync.dma_start(g[f"ag2_in{cch}h"][:, :, :], x2h[:, :, :])
                nc.sync.dma_start(g[f"ag2_in{cch}l"][:, :, :], x2l[:, :, :])
                for sfx in ("h", "l"):
                    nc.gpsimd.collective_compute(
                        "AllGather", mybir.AluOpType.bypass, replica_groups=rg,
                        ins=[g[f"ag2_in{cch}{sfx}"].opt()],
                        outs=[g[f"ag2_out{cch}{sfx}"].opt()],
                    )
            else:
                nc.sync.dma_start(g[f"ag2_out{cch}h"][0:128, :, :], x2h[:, :, :])
                nc.sync.dma_start(g[f"ag2_out{cch}l"][0:128, :, :], x2l[:, :, :])
            # token half 0 (source cores 0-3) -> SBUF in cp-major layout,
            # off the Pool DMA queue so it rides during o_proj without
            # blocking SP
            xh_t = g["xnha"] if off < 26 else g["xnhb"]
            xl_t = g["xnla"] if off < 26 else g["xnlb"]
            loff = off if off < 26 else off - 26
            for cp in range(4):
                nc.gpsimd.dma_start(
                    xh_t[:, cp, loff:loff + sz, :],
                    g[f"ag2_out{cch}h"][cp * 128:(cp + 1) * 128, :, :],
                )
                nc.gpsimd.dma_start(
                    xl_t[:, cp, loff:loff + sz, :],
                    g[f"ag2_out{cch}l"][cp * 128:(cp + 1) * 128, :, :],
                )

        for cch, (off, sz) in enumerate(AG_CHUNKS):
            # partial production for this chunk's m-range
            for mi in range(sz):
                m = off + mi
                ot = p3s.tile([128, S], BF16, name="ot3", tag="ot3", bufs=2)
                for cb in range(4):
                    ccols = slice(cb * 512, (cb + 1) * 512)
                    o_ps = p3p.tile([128, 512], F32, name="o_ps", tag="s_ps", bufs=4)
                    _dr3(nc, o_ps, wosh, wosl, ctxh, ctxl, QH,
                         ncols=ccols, first=True, last=True, wmid=m)
                    # partials stay at the 2^6 weight scale in bf16
                    nc.vector.tensor_copy(ot[:, ccols], o_ps[:])
                    if with_collectives:
                        for half in range(2):
                            c = cb * 2 + half
                            nc.sync.dma_start(
                                g[f"rsp_in{cch}"][c * 128:(c + 1) * 128, mi, :],
                                ot[:, cb * 512 + half * 256: cb * 512 + (half + 1) * 256])
                    elif cb == 0:
                        nc.sync.dma_start(
                            g[f"rsp_in{cch}"][0:128, mi, :], ot[:, 0:TPC])
            if with_collectives:
                nc.gpsimd.collective_compute(
                    "ReduceScatter", mybir.AluOpType.add, replica_groups=rg,
                    ins=[g[f"rsp_in{cch}"].opt()], outs=[g[f"rsp_out{cch}"].opt()],
                )
            else:
                nc.sync.dma_start(g[f"rsp_out{cch}"][:, :, :],
                                  g[f"rsp_in{cch}"][0:128, :, :])
            rsbs.append(rsb_load(cch, off, sz))
            if cch > 0:
                consumer(cch - 1, *AG_CHUNKS[cch - 1], rsbs[cch - 1])
        consumer(len(AG_CHUNKS) - 1, *AG_CHUNKS[-1], rsbs[-1])
        # ship the raw sum-of-squares row; every core derives rstd locally
        sums_sb = p3s.tile([1, TPC], F32, name="sums_sb", tag="sums_sb")
        nc.vector.tensor_copy(sums_sb[:], st2_ps[:])
        if with_collectives:
            nc.sync.dma_start(g["sums_in"][:, :], sums_sb[:])
            nc.gpsimd.collective_compute(
                "AllGather", mybir.AluOpType.bypass, replica_groups=rg,
                ins=[g["sums_in"].opt()], outs=[g["sums_out"].opt()],
            )
        else:
            nc.sync.dma_start(g["sums_out"][0:1, :], sums_sb[:])


def _phase45_mlp(nc, tc, g, with_collectives, rg):
    """Fused SwiGLU + down projection, one token half at a time.
    h never leaves SBUF; the 1/rms factor of rmsnorm2 (with the fp8 weight
    descale folded in) is applied to the gate/up psums.  ReduceScatter
    chunks (by output-row group) fire as the second half completes them."""
    with (
        tc.tile_pool(name="p45w", bufs=1) as pw,
        tc.tile_pool(name="p45big", bufs=1) as pb45,
        tc.tile_pool(name="p45sbuf", bufs=2) as ps,
    ):
        pp = g["psum"]
        hh8 = pb45.tile([128, KI, SH], FP8, name="hh8")
        hl8 = pb45.tile([128, KI, SH], FP8, name="hl8")
        # rstd for all tokens from the gathered sum-of-squares rows; the
        # Sqrt scale/bias are pre-multiplied by 2^12 so the reciprocal
        # comes out as rstd * 2^-6 (the fp8 weight descale rides along)
        sums8 = pw.tile([8, TPC], F32, name="sums8")
        nc.sync.dma_start(sums8[:], g["sums_out"][:, :])
        std8 = pw.tile([8, TPC], F32, name="std8")
        nc.scalar.activation(std8[:], sums8[:], AF.Sqrt,
                             bias=g["epsb8"][:], scale=SW * SW / H)
        rstd8 = pw.tile([8, TPC], F32, name="rstd8")
        nc.vector.reciprocal(rstd8[:], std8[:])
        nc.sync.dma_start(g["rstd_dram"][0, :, :], rstd8[:])
        rstd_row = g["rstd1_sb"]   # rmsnorm1's row tile, dead after phase 1
        nc.sync.dma_start(rstd_row[:], g["rstd_dram"][0:1, :, :])

        xnha, xnla = g["xnha"], g["xnla"]
        xnhb, xnlb = g["xnhb"], g["xnlb"]
        for T in range(2):
            tsl = slice(T * SH, (T + 1) * SH)
            rbh = ps.tile([128, SH], F32, name="rbh", tag="rbh", bufs=1)
            nc.gpsimd.partition_broadcast(rbh[:], rstd_row[:, tsl])
            # gate/up: 256-token column blocks (one gathered source core per
            # block, so the cp-major xn layout keeps 3D matmul APs)
            for m in range(MB_GU):
                if T == 0 and m == 0:
                    gbh, ubh = g["gb0h"], g["ub0h"]   # preloaded during o_proj
                    gbl, ubl = g["gb0l"], g["ub0l"]
                else:
                    gbh = ps.tile([128, KH, 128], FP8, name="gbh", tag="wgu", bufs=6)
                    nc.sync.dma_start(gbh[:], g["wguh"][:, m, :, :])
                    gbl = ps.tile([128, KH, 128], FP8, name="gbl", tag="wgu", bufs=6)
                    nc.sync.dma_start(gbl[:], g["wgul"][:, m, :, :])
                    ubh = ps.tile([128, KH, 128], FP8, name="ubh", tag="wgu", bufs=6)
                    nc.sync.dma_start(ubh[:], g["wguh"][:, MB_GU + m, :, :])
                    ubl = ps.tile([128, KH, 128], FP8, name="ubl", tag="wgu", bufs=6)
                    nc.sync.dma_start(ubl[:], g["wgul"][:, MB_GU + m, :, :])
                for tb in range(4):
                    lcols = slice(tb * 256, (tb + 1) * 256)
                    g_ps = pp.tile([128, 256], F32, name="g_ps", tag="s_ps", bufs=4)
                    _dr3(nc, g_ps, gbh, gbl, xnha, xnla, 26, first=True,
                         last=False, xmid=tb)
                    _dr3(nc, g_ps, gbh, gbl, xnhb, xnlb, 6, wk0=26,
                         first=False, last=True, xmid=tb)
                    u_ps = pp.tile([128, 256], F32, name="u_ps", tag="s_ps", bufs=4)
                    _dr3(nc, u_ps, ubh, ubl, xnha, xnla, 26, first=True,
                         last=False, xmid=tb)
                    _dr3(nc, u_ps, ubh, ubl, xnhb, xnlb, 6, wk0=26,
                         first=False, last=True, xmid=tb)
                    gsc = ps.tile([128, 256], F32R, name="gsc", tag="gsc", bufs=2)
                    nc.vector.tensor_mul(gsc[:], g_ps[:], rbh[:, lcols])
                    usc = ps.tile([128, 256], F32R, name="usc", tag="usc", bufs=2)
                    nc.vector.tensor_mul(usc[:], u_ps[:], rbh[:, lcols])
                    sg = ps.tile([128, 256], F32R, name="sg", tag="sg", bufs=2)
                    nc.scalar.activation(sg[:], gsc[:], AF.Silu)
                    h32 = ps.tile([128, 256], F32R, name="h32", tag="h32", bufs=2)
                    nc.vector.tensor_mul(h32[:], sg[:], usc[:])
                    nc.vector.tensor_copy(hh8[:, m, lcols], h32[:])
                    nc.vector.tensor_sub(hl8[:, m, lcols], h32[:], hh8[:, m, lcols])
            if T == 0:
                # second token half of the gathered activations: issue now so
                # they stream during the first half's down projection (the
                # WAR on xn resolved when the last gate/up matmul above ran)
                for cch, (off, sz) in enumerate(AG_CHUNKS):
                    xh_t = xnha if off < 26 else xnhb
                    xl_t = xnla if off < 26 else xnlb
                    loff = off if off < 26 else off - 26
                    for cp in range(4):
                        nc.gpsimd.dma_start(
                            xh_t[:, cp, loff:loff + sz, :],
                            g[f"ag2_out{cch}h"][(4 + cp) * 128:(5 + cp) * 128, :, :],
                        )
                        nc.gpsimd.dma_start(
                            xl_t[:, cp, loff:loff + sz, :],
                            g[f"ag2_out{cch}l"][(4 + cp) * 128:(5 + cp) * 128, :, :],
                        )
            if T == 1:
                # residual output, deferred into the last down stretch
                for cch, (off, sz) in enumerate(AG_CHUNKS):
                    nc.sync.dma_start(g["res_out"][:, off:off + sz, :],
                                      g[f"x2b{cch}"][:])
            # down
            for r in range(8):
                for mi in range(KH // 8):
                    m = r * (KH // 8) + mi
                    dbh = ps.tile([128, KI, 128], FP8, name="dbh", tag="db", bufs=4)
                    nc.sync.dma_start(dbh[:], g["wdnh"][:, m, :, :])
                    dbl = ps.tile([128, KI, 128], FP8, name="dbl", tag="db", bufs=4)
                    nc.sync.dma_start(dbl[:], g["wdnl"][:, m, :, :])
                    ot = ps.tile([128, SH], BF16, name="ot", tag="ot", bufs=2)
                    for tb in range(2):
                        lcols = slice(tb * 512, (tb + 1) * 512)
                        d_ps = pp.tile([128, 512], F32, name="d_ps", tag="mm", bufs=2)
                        # KI=14 k-tiles -> 7 DoubleRow pairs
                        _dr3(nc, d_ps, dbh, dbl, hh8, hl8, KI, ncols=lcols,
                             first=True, last=True)
                        # descale on the psum copy (ACT, off the DVE)
                        nc.scalar.activation(ot[:, lcols], d_ps[:], AF.Copy,
                                             scale=DSW)
                    nc.sync.dma_start(g[f"rs_in{r}"][mi * 128:(mi + 1) * 128, tsl], ot[:])
                if T == 1:
                    if with_collectives:
                        nc.gpsimd.collective_compute(
                            "ReduceScatter", mybir.AluOpType.add, replica_groups=rg,
                            ins=[g[f"rs_in{r}"].opt()], outs=[g[f"rs_out{r}"].opt()],
                        )
                    else:
                        nc.sync.dma_start(g[f"rs_out{r}"][:, :],
                                          g[f"rs_in{r}"][0:H // NC // 8, :])
                    # upconvert the bf16 shard to the fp32 output
                    ob = ps.tile([64, S], BF16, name="ob", tag="ob", bufs=1)
                    nc.gpsimd.dma_start(ob[:], g[f"rs_out{r}"][:, :])
                    for hf in range(4):
                        hsl2 = slice(hf * 512, (hf + 1) * 512)
                        of = ps.tile([64, 512], F32, name="of", tag="of", bufs=2)
                        nc.vector.tensor_copy(of[:], ob[:, hsl2])
                        nc.gpsimd.dma_start(g["out_down"][r * 64:(r + 1) * 64, hsl2], of[:])


def build_program(with_collectives=True, stop_after=99):
    nc = bacc.Bacc("TRN2", target_bir_lowering=False, debug=False, num_devices=NC)

    g = {}
    g["hTbh"] = nc.dram_tensor("hTbh", [128, KH, S], FP8, kind="ExternalInput")
    g["hTbl"] = nc.dram_tensor("hTbl", [128, KH, S], FP8, kind="ExternalInput")
    g["hT_slice"] = nc.dram_tensor("hT_slice", [128, KH, TPC], BF16, kind="ExternalInput")
    g["wqkvh"] = nc.dram_tensor("wqkvh", [128, QH + 2, KH, 128], FP8, kind="ExternalInput")
    g["wqkvl"] = nc.dram_tensor("wqkvl", [128, QH + 2, KH, 128], FP8, kind="ExternalInput")
    g["woh"] = nc.dram_tensor("woh", [128, KH, QH, 128], FP8, kind="ExternalInput")
    g["wol"] = nc.dram_tensor("wol", [128, KH, QH, 128], FP8, kind="ExternalInput")
    g["wguh"] = nc.dram_tensor("wguh", [128, 2 * MB_GU, KH, 128], FP8, kind="ExternalInput")
    g["wgul"] = nc.dram_tensor("wgul", [128, 2 * MB_GU, KH, 128], FP8, kind="ExternalInput")
    g["wdnh"] = nc.dram_tensor("wdnh", [128, KH, KI, 128], FP8, kind="ExternalInput")
    g["wdnl"] = nc.dram_tensor("wdnl", [128, KH, KI, 128], FP8, kind="ExternalInput")
    g["cosT"] = nc.dram_tensor("cosT", [128, S], BF16, kind="ExternalInput")
    g["sinT"] = nc.dram_tensor("sinT", [128, S], BF16, kind="ExternalInput")
    g["rstd1"] = nc.dram_tensor("rstd1", [1, S], F32, kind="ExternalInput")
    g["masks"] = nc.dram_tensor("masks", [128, 4, 512], BF16, kind="ExternalInput")

    g["res_out"] = nc.dram_tensor("res_out", [128, KH, TPC], BF16, kind="ExternalOutput")
    g["out_down"] = nc.dram_tensor("out_down", [H // NC, S], F32, kind="ExternalOutput")

    rg = [list(range(NC))]

    with tile.TileContext(nc) as tc:
        with (
            tc.tile_pool(name="consts", bufs=1) as consts,
            tc.tile_pool(name="dram", bufs=1, space="DRAM") as dram,
        ):
            shr = {"addr_space": "Shared"} if with_collectives else {}
            for cch, (off, sz) in enumerate(AG_CHUNKS):
                g[f"rsp_in{cch}"] = dram.tile([NC * 128, sz, TPC], BF16,
                                              name=f"rsp_in{cch}")
                g[f"rsp_out{cch}"] = dram.tile([128, sz, TPC], BF16,
                                               name=f"rsp_out{cch}")
                for sfx in ("h", "l"):
                    g[f"ag2_in{cch}{sfx}"] = dram.tile([128, sz, TPC], FP8,
                                                       name=f"ag2_in{cch}{sfx}")
                    g[f"ag2_out{cch}{sfx}"] = dram.tile([NC * 128, sz, TPC], FP8,
                                                        name=f"ag2_out{cch}{sfx}", **shr)
            g["sums_in"] = dram.tile([1, TPC], F32, name="sums_in")
            g["sums_out"] = dram.tile([NC, TPC], F32, name="sums_out", addr_space="Shared")
            g["rstd_dram"] = dram.tile([1, NC, TPC], F32, name="rstd_dram")
            for r in range(8):
                g[f"rs_in{r}"] = dram.tile([H // 8, S], BF16, name=f"rs_in{r}")
                g[f"rs_out{r}"] = dram.tile([H // NC // 8, S], BF16, name=f"rs_out{r}")

            ones32 = consts.tile([128, 1], F32, name="ones32")
            nc.gpsimd.memset(ones32[:], 1.0)
            g["ones"] = consts.tile([128, 1], F32R, name="ones")
            nc.vector.tensor_copy(g["ones"][:], ones32[:])
            g["ones_b"] = consts.tile([128, 1], BF16, name="ones_b")
            nc.vector.tensor_copy(g["ones_b"][:], ones32[:])
            ident32 = consts.tile([128, 128], F32, name="ident32")
            make_identity(nc, ident32[:])
            g["ident"] = consts.tile([128, 128], BF16, name="ident")
            nc.vector.tensor_copy(g["ident"][:], ident32[:])
            g["epsb8"] = consts.tile([8, 1], F32, name="epsb8")
            nc.gpsimd.memset(g["epsb8"][:], EPS * SW * SW)
            g["rstd1_sb"] = consts.tile([1, S], F32, name="rstd1_sb")

            # reserved early: hsl preloads during the DMA-quiet qkv window;
            # the fp8 hi/lo context tiles are written during attention and
            # contracted by the row-parallel o_proj in phase 3
            psum_cm = tc.tile_pool(name="psum", bufs=1, space="PSUM")
            g["psum"] = psum_cm.__enter__()
            wop_cm = tc.tile_pool(name="wopre", bufs=1)
            wop = wop_cm.__enter__()
            g["hsl_sb"] = wop.tile([128, KH, TPC], BF16, name="hsl_sb")
            g["ctxh_sb"] = wop.tile([128, QH, S], FP8, name="ctxh_sb")
            g["ctxl_sb"] = wop.tile([128, QH, S], FP8, name="ctxl_sb")

            with tc.tile_pool(name="attn", bufs=1) as attn:
                g["mask_sb"] = attn.tile([128, 4, 512], BF16, name="mask_sb")
                g["qT_sb"] = attn.tile([128, QH, S], BF16, name="qT_sb")          # 2 MB
                g["kT_sb"] = attn.tile([128, S], BF16, name="kT_sb")              # 0.5 MB
                g["v_tok"] = attn.tile([128, S // 128, 128], BF16, name="v_tok")  # 0.5 MB

                with tc.tile_pool(name="p2sbuf", bufs=2) as p2s:
                    _phase12_qkv_attn(nc, tc, g, p2s, with_collectives, rg)

            if stop_after >= 3:
                with tc.tile_pool(name="mlpbig", bufs=1) as pb:
                    g["xnha"] = pb.tile([128, 4, 26, TPC], FP8, name="xnha")
                    g["xnla"] = pb.tile([128, 4, 26, TPC], FP8, name="xnla")
                    g["xnhb"] = pb.tile([128, 4, 6, TPC], FP8, name="xnhb")
                    g["xnlb"] = pb.tile([128, 4, 6, TPC], FP8, name="xnlb")
                    g["gb0h"] = pb.tile([128, KH, 128], FP8, name="gb0h")
                    g["gb0l"] = pb.tile([128, KH, 128], FP8, name="gb0l")
                    g["ub0h"] = pb.tile([128, KH, 128], FP8, name="ub0h")
                    g["ub0l"] = pb.tile([128, KH, 128], FP8, name="ub0l")
                    _phase3_oproj(nc, tc, g, with_collectives, rg)
                    if stop_after >= 4:
                        _phase45_mlp(nc, tc, g, with_collectives, rg)
            wop_cm.__exit__(None, None, None)
            psum_cm.__exit__(None, None, None)

    nc.finalize()
    return nc


_cached_nc = None


def _get_nc():
    global _cached_nc
    if _cached_nc is None:
        _cached_nc = build_program(with_collectives=True)
    return _cached_nc


def _split8(a):
    """fp8 e4m3 hi/lo split of a float32 array."""
    hi = a.astype(E4NP)
    lo = (a - hi.astype(np.float32)).astype(E4NP)
    return hi, lo


def _host_prep(positions, hidden_states, w_qkv, w_o, w_gate_up, w_down, ln1_w, ln2_w):
    f32 = np.float32
    bf16 = ml_dtypes.bfloat16
    hidden = np.asarray(hidden_states, dtype=f32)[0]          # [S, H]
    hT = np.ascontiguousarray(hidden.T)                        # [H, S]
    hTb_np = np.ascontiguousarray(
        hT.reshape(KH, 128, S).transpose(1, 0, 2))             # [128, KH, S] f32
    hTbh_np, hTbl_np = _split8(hTb_np)
    pos = np.asarray(positions).astype(f32)[0]                 # [S]

    half = HD // 2
    inv_freq = (1.0 / (f32(THETA) ** (np.arange(0, half, dtype=f32) / f32(half)))).astype(f32)
    ang = pos[:, None] * inv_freq[None, :]                     # [S, 64] fp32
    # rmsnorm1 only depends on the input: precompute 1/rms per token and
    # fold it (and the fp8 weight descale 2^-6) into the rope tables (both
    # commute through the QKV matmul)
    rstd1_np = (1.0 / np.sqrt((hidden.astype(np.float64) ** 2).mean(axis=1) + EPS)
                ).astype(f32)                                  # [S]
    cos_half = (np.cos(ang).astype(f32) * (rstd1_np * f32(DSW))[:, None]).T  # [64, S]
    sin_half = (np.sin(ang).astype(f32) * (rstd1_np * f32(DSW))[:, None]).T
    cosT_np = np.ascontiguousarray(np.concatenate([cos_half, cos_half], axis=0)).astype(bf16)  # [128, S]
    sinT_np = np.ascontiguousarray(np.concatenate([sin_half, sin_half], axis=0)).astype(bf16)
    rstd1_row = np.ascontiguousarray((rstd1_np * f32(DSW)).reshape(1, S))

    w_qkv_f = np.asarray(w_qkv, dtype=f32) * np.asarray(ln1_w, dtype=f32)[:, None] * f32(SW)
    w_gu_f = np.asarray(w_gate_up, dtype=f32) * np.asarray(ln2_w, dtype=f32)[:, None] * f32(SW)
    w_o_f32 = np.asarray(w_o, dtype=f32) * f32(SW)             # [NQ*HD, H]
    w_dn_f = np.asarray(w_down, dtype=f32) * f32(SW)

    kk = np.arange(128)[:, None, None]
    jj = np.arange(4)[None, :, None]
    qq = np.arange(512)[None, None, :]
    masks_np = np.ascontiguousarray((qq >= kk + 128 * jj).astype(bf16))  # [128, 4, 512]

    in_maps = []
    for c in range(NC):
        # o_proj row slice for this core's 4 heads -> [128, m, head_kt, 128]
        wo_c = np.ascontiguousarray(
            w_o_f32[c * QH * HD:(c + 1) * QH * HD, :]
            .reshape(QH, 128, KH, 128).transpose(1, 2, 0, 3))
        woh_c, wol_c = _split8(wo_c)
        q_cols = w_qkv_f[:, c * QH * HD:(c + 1) * QH * HD]
        k_col = w_qkv_f[:, NQ * HD + c * HD: NQ * HD + (c + 1) * HD]
        v_col = w_qkv_f[:, (NQ + NKV) * HD + c * HD: (NQ + NKV) * HD + (c + 1) * HD]
        wqkv_c = np.concatenate([q_cols, k_col, v_col], axis=1)
        wqkv_c = np.ascontiguousarray(
            wqkv_c.reshape(KH, 128, QH + 2, 128).transpose(1, 2, 0, 3))
        wqkvh_c, wqkvl_c = _split8(wqkv_c)
        wgu_c = np.concatenate(
            [w_gu_f[:, c * IPC:(c + 1) * IPC],
             w_gu_f[:, I + c * IPC: I + (c + 1) * IPC]], axis=1)
        wgu_c = np.ascontiguousarray(
            wgu_c.reshape(KH, 128, 2 * MB_GU, 128).transpose(1, 2, 0, 3))
        wguh_c, wgul_c = _split8(wgu_c)
        wdn_c = np.ascontiguousarray(
            w_dn_f[c * IPC:(c + 1) * IPC, :].reshape(KI, 128, KH, 128).transpose(1, 2, 0, 3))
        wdnh_c, wdnl_c = _split8(wdn_c)
        hT_slice_c = np.ascontiguousarray(
            hT[:, c * TPC:(c + 1) * TPC].reshape(KH, 128, TPC).transpose(1, 0, 2)
        ).astype(bf16)
        in_maps.append({
            "hTbh": hTbh_np,
            "hTbl": hTbl_np,
            "hT_slice": hT_slice_c,
            "wqkvh": wqkvh_c,
            "wqkvl": wqkvl_c,
            "woh": woh_c,
            "wol": wol_c,
            "wguh": wguh_c,
            "wgul": wgul_c,
            "wdnh": wdnh_c,
            "wdnl": wdnl_c,
            "cosT": cosT_np,
            "sinT": sinT_np,
            "rstd1": rstd1_row,
            "masks": masks_np,
        })
    return in_maps


def kernel(**inputs):
    in_maps = _host_prep(**inputs)
    nc = _get_nc()
    res = run_bass_kernel_spmd(nc, in_maps, core_ids=list(range(NC)))
    results = res.results

    outT = np.empty((H, S), np.float32)
    for c in range(NC):
        od = results[c]["out_down"]           # [512, S]: chunk r rows -> global 512r+64c
        for r in range(8):
            outT[512 * r + 64 * c: 512 * r + 64 * (c + 1)] = od[64 * r:64 * (r + 1)]
    resT = np.concatenate(
        [np.asarray(results[c]["res_out"]).astype(np.float32)
         .transpose(1, 0, 2).reshape(H, TPC)
         for c in range(NC)], axis=1)          # [H, S]
    out = np.ascontiguousarray(outT.T).reshape(1, S, H).astype(np.float32)
    residual = np.ascontiguousarray(resT.T).reshape(1, S, H).astype(np.float32)
    return out, residual


# revision 61
# speedup vs baseline: 1.0170x; 1.0170x over previous
"""Bamba attention decoder layer on 8 Trainium2 NeuronCores.

Sharding: tensor-parallel attention (4 q heads + 1 kv head per core),
AllToAll of attention context, token-sliced o_proj + fused add, chunked
AllGather of the *unnormalized* residual (the 1/rms factor commutes through
the gate/up matmuls and is applied on the consumer side), I-sharded SwiGLU
MLP (1792 cols/core) fused with the down projection per token half (h stays
in SBUF), ReduceScatter of down-proj partials.

All four projections (qkv, o_proj, gate_up, down) run as split-precision
fp8e4 DoubleRow matmuls: each operand X is represented as X_hi + X_lo (both
e4m3), and the product keeps the three dominant cross terms
Wh*Xh + Wh*Xl + Wl*Xh (the dropped Wl*Xl term is ~0.07%).  A DoubleRow
matmul contracts two 128-deep k-tiles per instruction at 0.5 cycles/row, so
the three terms per 256-K pair cost 0.75x the bf16 cycles while keeping
~bf16-level accuracy.  Weights are pre-scaled by 2^6 on the host (their
sigma=0.02 values would otherwise land in the fp8 subnormal range); the
2^-6 descale folds into the rope tables, the rstd rows, the fused
residual-add and the down-proj psum copy.  Attention stays bf16 (exp values
overflow fp8).  The residual stream, rmsnorm statistics and kernel outputs
stay fp32/bf16.
"""

import numpy as np
import ml_dtypes

import concourse.bacc as bacc
import concourse.bass_isa as bass_isa
import concourse.mybir as mybir
import concourse.tile as tile
from concourse.bass_utils import run_bass_kernel_spmd
from concourse.masks import make_identity

NC = 8
S = 2048
H = 4096
HD = 128
NQ = 32
NKV = 8
I = 14336
QH = NQ // NC        # q heads per core = 4
IPC = I // NC        # intermediate cols per core = 1792
TPC = S // NC        # tokens per core = 256
EPS = 1e-5
THETA = 10000.0
SCALE = HD ** -0.5
SW = 2.0 ** 6        # host-side weight scale (fp8 subnormal avoidance)
DSW = 2.0 ** -6

F32 = mybir.dt.float32
F32R = mybir.dt.float32r
BF16 = mybir.dt.bfloat16
FP8 = mybir.dt.float8e4
E4NP = ml_dtypes.float8_e4m3

KH = H // 128        # 32 k-tiles over H
NB = S // 512        # 4 token blocks of 512
MB_GU = IPC // 128   # 14 m tiles for gate (and for up)
KI = IPC // 128      # 14 k tiles over I per core
SH = S // 2          # tokens per half = 1024
# AllGather chunking of the residual stream: smaller tail chunks so the
# last chunk's transfer chain at the o_proj->MLP boundary is short
AG_CHUNKS = [(0, 8), (8, 8), (16, 6), (22, 4), (26, 4), (30, 2)]

AF = mybir.ActivationFunctionType
DR = mybir.MatmulPerfMode.DoubleRow
ALU = mybir.AluOpType


def _dr3(nc, out_ps, wh, wl, xh, xl, nkt, wk0=0, ncols=None, first=False,
         last=False, wmid=None, xmid=None, skip_gc=False):
    """Accumulate the 3-term split-fp8 product over k-tile pairs into
    out_ps: Wh*Xh + Wh*Xl + Wl*Xh per pair (the Wl*Xl term is dropped).
    xh/xl are indexed 0..nkt; the weight k-tiles start at wk0.  nkt must be
    even.  ncols optionally slices the moving columns.  wmid/xmid index an
    extra leading dim for 4D [128, mid, kt, c] tiles."""
    terms = []
    for t in range(0, nkt, 2):
        wsl = slice(wk0 + t, wk0 + t + 2)
        xsl = slice(t, t + 2)
        terms.append((wh, xh, wsl, xsl))
        terms.append((wh, xl, wsl, xsl))
        terms.append((wl, xh, wsl, xsl))
    n = len(terms)
    for i, (a, b, wsl, xsl) in enumerate(terms):
        lhsT = a[:, wsl, :] if wmid is None else a[:, wmid, wsl, :]
        csl = slice(None) if ncols is None else ncols
        rhs = b[:, xsl, csl] if xmid is None else b[:, xmid, xsl, csl]
        nc.tensor.matmul(
            out_ps[:], lhsT, rhs,
            start=(first and i == 0), stop=(last and i == n - 1),
            perf_mode=DR, skip_group_check=skip_gc,
        )


def _attn_block(nc, g, p2s, p2p, hh, qb, with_collectives):
    """Causal GQA attention for head hh, q-block qb (512 q tokens)."""
    nkt = 4 * qb + 4
    att_ps = p2p.tile([128, 512], F32, name="att_ps", tag="att_ps", bufs=1)
    sums_ps = p2p.tile([1, 512], F32, name="sums_ps", tag="sums_ps", bufs=1)
    # waves: issue score matmuls back-to-back, then their att/sums
    # accumulations — by the time the PE (in-order) reaches an att matmul,
    # its exp+mask chain has drained, so it doesn't bubble per tile
    for w0 in range(0, nkt, 16):
        wave = range(w0, min(w0 + 16, nkt))
        es = []
        for kt in wave:
            j = kt - 4 * qb
            # on diagonal tiles only q-columns >= 128*j attend at all;
            # skip the fully-masked column range entirely
            q0 = 128 * j if j > 0 else 0
            lsl = slice(q0, 512)
            s_ps = p2p.tile([128, 512], F32, name="s_ps", tag="s_ps", bufs=4)
            nc.tensor.matmul(
                s_ps[:, lsl], g["kT_sb"][:, kt * 128:(kt + 1) * 128],
                g["qT_sb"][:, hh, qb * 512 + q0:(qb + 1) * 512],
                start=True, stop=True,
            )
            e = p2s.tile([128, 512], BF16, name="e", tag="e", bufs=8)
            nc.scalar.activation(e[:, lsl], s_ps[:, lsl], AF.Exp, scale=SCALE)
            if j >= 0:
                nc.vector.tensor_mul(e[:, lsl], e[:, lsl], g["mask_sb"][:, j, lsl])
            es.append((kt, lsl, e))
        for kt, lsl, e in es:
            nc.tensor.matmul(att_ps[:, lsl], g["v_tok"][:, kt, :], e[:, lsl],
                             start=(kt == 0), stop=(kt == nkt - 1),
                             skip_group_check=True)
            nc.tensor.matmul(sums_ps[:, lsl], g["ones_b"][:], e[:, lsl],
                             start=(kt == 0), stop=(kt == nkt - 1),
                             skip_group_check=True)
    recip = p2s.tile([1, 512], F32, name="recip", tag="recip")
    nc.vector.reciprocal(recip[:], sums_ps[:])
    rb2 = p2s.tile([128, 512], F32, name="rb2", tag="rb2", bufs=1)
    nc.gpsimd.partition_broadcast(rb2[:], recip[:])
    # split the normalized context into fp8 hi+lo straight into the
    # SBUF-resident context tiles the row-parallel o_proj contracts over
    an32 = p2s.tile([128, 512], F32R, name="an32", tag="an32", bufs=1)
    nc.vector.tensor_mul(an32[:], att_ps[:], rb2[:])
    csl = slice(qb * 512, (qb + 1) * 512)
    nc.vector.tensor_copy(g["ctxh_sb"][:, hh, csl], an32[:])
    nc.vector.tensor_sub(g["ctxl_sb"][:, hh, csl], an32[:],
                         g["ctxh_sb"][:, hh, csl])


def _phase12_qkv_attn(nc, tc, g, p2s, with_collectives, rg):
    p1p = g["psum"]
    """QKV matmul + rope, fused with attention: the attention for token
    block nb runs right after block nb's rope, filling the PE while the
    exp/softmax pipeline of earlier blocks drains.  The rmsnorm1 factor is
    precomputed on the host (it only depends on the kernel input) and folded
    into cosT/sinT; v is scaled by the hosted rstd1 row directly."""
    with (
        tc.tile_pool(name="p1sbuf", bufs=2) as p1s,
        tc.tile_pool(name="p1w", bufs=1) as p1w,
    ):
        # first-needed-first DMA order: block-0 activations + first weight
        # chunk go ahead of everything else so the PE starts within ~5us.
        def load_block(nb):
            ncols = slice(nb * 512, (nb + 1) * 512)
            hhs, hls = [], []
            for kc in range(4):
                hh_ = p1s.tile([128, 8, 512], FP8, name="hbh", tag="hbh", bufs=6)
                nc.sync.dma_start(hh_[:], g["hTbh"][:, kc * 8:(kc + 1) * 8, ncols])
                hhs.append(hh_)
                hl_ = p1s.tile([128, 8, 512], FP8, name="hbl", tag="hbl", bufs=6)
                nc.sync.dma_start(hl_[:], g["hTbl"][:, kc * 8:(kc + 1) * 8, ncols])
                hls.append(hl_)
            return hhs, hls

        hb_h, hb_l = [], []
        for kc in range(4):
            hb_h.append(p1s.tile([128, 8, 512], FP8, name="hbh", tag="hbh", bufs=6))
            hb_l.append(p1s.tile([128, 8, 512], FP8, name="hbl", tag="hbl", bufs=6))
        wq_h, wq_l = [], []
        for m in range(QH + 2):
            wq_h.append(p1w.tile([128, KH, 128], FP8, name=f"wqh{m}"))
            wq_l.append(p1w.tile([128, KH, 128], FP8, name=f"wql{m}"))
        # m-major weight chunks interleaved with block-0 activations: m0's
        # first weights + the first activation chunk arrive within ~2us
        nc.sync.dma_start(hb_h[0][:, 0:4, :], g["hTbh"][:, 0:4, 0:512])
        nc.sync.dma_start(wq_h[0][:], g["wqkvh"][:, 0, :, :])
        nc.sync.dma_start(hb_l[0][:, 0:4, :], g["hTbl"][:, 0:4, 0:512])
        nc.sync.dma_start(wq_l[0][:], g["wqkvl"][:, 0, :, :])
        nc.sync.dma_start(hb_h[0][:, 4:8, :], g["hTbh"][:, 4:8, 0:512])
        nc.sync.dma_start(hb_l[0][:, 4:8, :], g["hTbl"][:, 4:8, 0:512])
        nc.sync.dma_start(wq_h[1][:], g["wqkvh"][:, 1, :, :])
        nc.sync.dma_start(wq_l[1][:], g["wqkvl"][:, 1, :, :])
        for kc in range(1, 4):
            nc.sync.dma_start(hb_h[kc][:], g["hTbh"][:, kc * 8:(kc + 1) * 8, 0:512])
            nc.sync.dma_start(hb_l[kc][:], g["hTbl"][:, kc * 8:(kc + 1) * 8, 0:512])
        cos_sb = p1w.tile([128, S], BF16, name="cos_sb")
        nc.sync.dma_start(cos_sb[:, 0:512], g["cosT"][:, 0:512])
        sin_sb = p1w.tile([128, S], BF16, name="sin_sb")
        nc.sync.dma_start(sin_sb[:, 0:512], g["sinT"][:, 0:512])
        rstd1 = g["rstd1_sb"]
        nc.sync.dma_start(rstd1[:], g["rstd1"][:, :])
        for m in range(2, QH + 2):
            nc.sync.dma_start(wq_h[m][:], g["wqkvh"][:, m, :, :])
            nc.sync.dma_start(wq_l[m][:], g["wqkvl"][:, m, :, :])
        nc.sync.dma_start(cos_sb[:, 512:S], g["cosT"][:, 512:S])
        nc.sync.dma_start(sin_sb[:, 512:S], g["sinT"][:, 512:S])
        nc.sync.dma_start(g["mask_sb"][:], g["masks"][:, :, :])

        nxt = load_block(1)   # double-buffered block prefetch
        for nb in range(NB):
            ncols = slice(nb * 512, (nb + 1) * 512)
            if nb == 0:
                hhs, hls = hb_h, hb_l
            else:
                hhs, hls = nxt
                nxt = load_block(nb + 1) if nb + 1 < NB else None
            if nb == 2:
                # hT_slice for the o_proj residual add: load it during the
                # DMA-quiet attention stretch, not o_proj's saturated window
                nc.sync.dma_start(g["hsl_sb"][:], g["hT_slice"][:, :, :])
            rb = p1s.tile([128, 512], F32, name="rb", tag="rb", bufs=2)
            nc.gpsimd.partition_broadcast(rb[:], rstd1[:, ncols])

            def finish_m(m, mm):
                if m < QH + 1:
                    qkc = p1s.tile([128, 512], F32, name="qkc", tag="qkc", bufs=1)
                    nc.scalar.copy(qkc[:], mm[:])
                    if m < QH:
                        d0 = g["qT_sb"][0:64, m, ncols]
                        d1 = g["qT_sb"][64:128, m, ncols]
                    else:
                        d0 = g["kT_sb"][0:64, ncols]
                        d1 = g["kT_sb"][64:128, ncols]
                    t0 = p1s.tile([64, 512], F32, name="t0", tag="t0", bufs=1)
                    nc.vector.tensor_mul(t0[:], qkc[0:64, :], cos_sb[0:64, ncols])
                    t1 = p1s.tile([64, 512], F32, name="t1", tag="t1", bufs=1)
                    nc.vector.tensor_mul(t1[:], qkc[64:128, :], sin_sb[64:128, ncols])
                    nc.vector.tensor_sub(d0, t0[:], t1[:])
                    t2 = p1s.tile([64, 512], F32, name="t2", tag="t0", bufs=1)
                    nc.vector.tensor_mul(t2[:], qkc[64:128, :], cos_sb[64:128, ncols])
                    t3 = p1s.tile([64, 512], F32, name="t3", tag="t1", bufs=1)
                    nc.vector.tensor_mul(t3[:], qkc[0:64, :], sin_sb[0:64, ncols])
                    nc.vector.tensor_add(d1, t2[:], t3[:])
                else:
                    vtmp = p1s.tile([128, 512], BF16, name="vtmp", tag="vtmp", bufs=1)
                    nc.vector.tensor_mul(vtmp[:], mm[:], rb[:])
                    for j in range(4):
                        # shares the attention-score psum tag (bank budget)
                        tp = p1p.tile([128, 128], BF16, name="tp", tag="s_ps",
                                      bufs=4)
                        nc.tensor.transpose(tp[:], vtmp[:, j * 128:(j + 1) * 128],
                                            g["ident"][:])
                        nc.vector.tensor_copy(g["v_tok"][:, nb * 4 + j, :], tp[:])

            # m-outer: one accumulating psum at a time (block-resident hb);
            # the previous block's attention interleaves between this
            # block's qkv chains, so each ACT-paced attention unit is
            # followed by a dense matmul chain and the in-order PE doesn't
            # idle on the exp pipeline
            if nb == 0:
                # kc-outer for the first block: the PE starts on activation
                # chunk 0 while chunks 1-3 are still streaming in (6 psum
                # groups stay open, one per output m-tile)
                # reuse the attention-phase psum tags (idle during block 0)
                mm_tags = [("s_ps", 4), ("s_ps", 4), ("s_ps", 4), ("s_ps", 4),
                           ("att_ps", 1), ("mm", 2)]
                mms = [p1p.tile([128, 512], F32, name=f"mm0_{m}", tag=tg,
                                bufs=bf) for m, (tg, bf) in enumerate(mm_tags)]
                for kc in range(4):
                    for m in range(QH + 2):
                        _dr3(nc, mms[m], wq_h[m], wq_l[m], hhs[kc], hls[kc], 8,
                             wk0=kc * 8, first=(kc == 0), last=(kc == 3),
                             skip_gc=True)
                for m in range(QH + 2):
                    finish_m(m, mms[m])
            else:
                for m in range(QH + 2):
                    mm = p1p.tile([128, 512], F32, name="mm", tag="mm", bufs=2)
                    for kc in range(4):
                        # hb tiles hold k-tiles kc*8..kc*8+8 at dim-1 index 0..8
                        _dr3(nc, mm, wq_h[m], wq_l[m], hhs[kc], hls[kc], 8,
                             wk0=kc * 8, first=(kc == 0), last=(kc == 3))
                    finish_m(m, mm)
                    if m < QH:
                        _attn_block(nc, g, p2s, p1p, m, nb - 1, with_collectives)
        # the last block's attention runs after its rope; its ctx writes
        # ship each head's context as it completes
        for hh in range(QH):
            _attn_block(nc, g, p2s, p1p, hh, NB - 1, with_collectives)


def _phase3_oproj(nc, tc, g, with_collectives, rg):
    """Row-parallel o_proj: each core contracts only its own 4 heads
    (K=512) against its w_o row slice and ReduceScatters bf16 partials
    (chunked by output k-tile range), receiving its 256-token slice summed.
    Then the fused descale + residual add, sum-of-squares, and the chunked
    AllGather of the unnormalized residual (fp8 hi+lo).  First-token-half
    xn loads ride the Pool DMA queue as gathered chunks land."""
    with (
        tc.tile_pool(name="p3sbuf", bufs=2) as p3s,
        tc.tile_pool(name="p3big", bufs=1) as p3b,
    ):
        p3p = g["psum"]
        # o_proj weights (own-head row slice, 4.2 MB)
        wosh = p3b.tile([128, KH, QH, 128], FP8, name="wosh")
        nc.sync.dma_start(wosh[:], g["woh"][:, :, :, :])
        wosl = p3b.tile([128, KH, QH, 128], FP8, name="wosl")
        nc.sync.dma_start(wosl[:], g["wol"][:, :, :, :])
        ctxh, ctxl = g["ctxh_sb"], g["ctxl_sb"]   # written during attention
        hsl = g["hsl_sb"]   # preloaded during phase 1
        st2_ps = p3p.tile([1, TPC], F32, name="st2_ps", tag="sums_ps", bufs=1)
        rsbs = []

        def rsb_load(cch, off, sz):
            rsb = p3s.tile([128, sz, TPC], BF16, name="rsb", tag="rsb", bufs=2)
            nc.sync.dma_start(rsb[:], g[f"rsp_out{cch}"][:, :, :])
            return rsb

        def consumer(cch, off, sz, rsb):
            """RS-result consumption for one chunk: fused residual add,
            sum-of-squares, bf16 + fp8 hi/lo copies, AllGather, xn loads.
            Deferred one chunk behind the partial matmuls so the RS round
            trip never stalls the in-order PE queue."""
            x2b = p3b.tile([128, sz, TPC], BF16, name=f"x2b{cch}")
            x2h = p3b.tile([128, sz, TPC], FP8, name=f"x2h{cch}")
            x2l = p3b.tile([128, sz, TPC], FP8, name=f"x2l{cch}")
            g[f"x2b{cch}"] = x2b
            for mi in range(sz):
                m = off + mi
                # fused descale + residual add: res2 = rs * 2^-6 + hsl
                res2 = p3s.tile([128, TPC], F32, name="res2", tag="res2", bufs=2)
                nc.vector.scalar_tensor_tensor(
                    res2[:], rsb[:, mi, :], DSW, hsl[:, m, :], ALU.mult, ALU.add)
                sq2 = p3s.tile([128, TPC], F32R, name="sq2", tag="sq2", bufs=2)
                nc.scalar.activation(sq2[:], res2[:], AF.Square)
                nc.tensor.matmul(st2_ps[:], g["ones"][:], sq2[:],
                                 start=(m == 0), stop=(m == KH - 1),
                                 skip_group_check=True)
                nc.vector.tensor_copy(x2b[:, mi, :], res2[:])
                nc.vector.tensor_copy(x2h[:, mi, :], res2[:])
                nc.vector.tensor_sub(x2l[:, mi, :], res2[:], x2h[:, mi, :])
                if m == 0:
                    # prefetch the first gate/up weights off the Pool
                    # queue so they're resident when the MLP starts
                    nc.gpsimd.dma_start(g["gb0h"][:], g["wguh"][:, 0, :, :])
                    nc.gpsimd.dma_start(g["gb0l"][:], g["wgul"][:, 0, :, :])
                    nc.gpsimd.dma_start(g["ub0h"][:], g["wguh"][:, MB_GU, :, :])
                    nc.gpsimd.dma_start(g["ub0l"][:], g["wgul"][:, MB_GU, :, :])
            # AllGather chain first (it feeds the MLP's critical path),
            # res_out write after.  Fallback writes the core's own slice
            # straight to ag2_out, skipping the staging copy.
            if with_collectives:
                nc.sync.dma_start(g[f"ag2_in{cch}h"][:, :, :], x2h[:, :, :])
                nc.sync.dma_start(g[f"ag2_in{cch}l"][:, :, :], x2l[:, :, :])
                for sfx in ("h", "l"):
                    nc.gpsimd.collective_compute(
                        "AllGather", mybir.AluOpType.bypass, replica_groups=rg,
                        ins=[g[f"ag2_in{cch}{sfx}"].opt()],
                        outs=[g[f"ag2_out{cch}{sfx}"].opt()],
                    )
            else:
                nc.sync.dma_start(g[f"ag2_out{cch}h"][0:128, :, :], x2h[:, :, :])
                nc.sync.dma_start(g[f"ag2_out{cch}l"][0:128, :, :], x2l[:, :, :])
            # token half 0 (source cores 0-3) -> SBUF in cp-major layout,
            # off the Pool DMA queue so it rides during o_proj without
            # blocking SP
            xh_t = g["xnha"] if off < 26 else g["xnhb"]
            xl_t = g["xnla"] if off < 26 else g["xnlb"]
            loff = off if off < 26 else off - 26
            for cp in range(4):
                nc.gpsimd.dma_start(
                    xh_t[:, cp, loff:loff + sz, :],
                    g[f"ag2_out{cch}h"][cp * 128:(cp + 1) * 128, :, :],
                )
                nc.gpsimd.dma_start(
                    xl_t[:, cp, loff:loff + sz, :],
                    g[f"ag2_out{cch}l"][cp * 128:(cp + 1) * 128, :, :],
                )

        for cch, (off, sz) in enumerate(AG_CHUNKS):
            # partial production for this chunk's m-range
            for mi in range(sz):
                m = off + mi
                ot = p3s.tile([128, S], BF16, name="ot3", tag="ot3", bufs=2)
                for cb in range(4):
                    ccols = slice(cb * 512, (cb + 1) * 512)
                    o_ps = p3p.tile([128, 512], F32, name="o_ps", tag="s_ps", bufs=4)
                    _dr3(nc, o_ps, wosh, wosl, ctxh, ctxl, QH,
                         ncols=ccols, first=True, last=True, wmid=m)
                    # partials stay at the 2^6 weight scale in bf16
                    nc.scalar.activation(ot[:, ccols], o_ps[:], AF.Copy)
                    if with_collectives:
                        for half in range(2):
                            c = cb * 2 + half
                            nc.sync.dma_start(
                                g[f"rsp_in{cch}"][c * 128:(c + 1) * 128, mi, :],
                                ot[:, cb * 512 + half * 256: cb * 512 + (half + 1) * 256])
                    elif cb == 0:
                        nc.sync.dma_start(
                            g[f"rsp_in{cch}"][0:128, mi, :], ot[:, 0:TPC])
            if with_collectives:
                nc.gpsimd.collective_compute(
                    "ReduceScatter", mybir.AluOpType.add, replica_groups=rg,
                    ins=[g[f"rsp_in{cch}"].opt()], outs=[g[f"rsp_out{cch}"].opt()],
                )
            else:
                nc.sync.dma_start(g[f"rsp_out{cch}"][:, :, :],
                                  g[f"rsp_in{cch}"][0:128, :, :])
            rsbs.append(rsb_load(cch, off, sz))
            if cch > 0:
                consumer(cch - 1, *AG_CHUNKS[cch - 1], rsbs[cch - 1])
        consumer(len(AG_CHUNKS) - 1, *AG_CHUNKS[-1], rsbs[-1])
        # ship the raw sum-of-squares row; every core derives rstd locally
        sums_sb = p3s.tile([1, TPC], F32, name="sums_sb", tag="sums_sb")
        nc.vector.tensor_copy(sums_sb[:], st2_ps[:])
        if with_collectives:
            nc.sync.dma_start(g["sums_in"][:, :], sums_sb[:])
            nc.gpsimd.collective_compute(
                "AllGather", mybir.AluOpType.bypass, replica_groups=rg,
                ins=[g["sums_in"].opt()], outs=[g["sums_out"].opt()],
            )
        else:
            nc.sync.dma_start(g["sums_out"][0:1, :], sums_sb[:])


def _phase45_mlp(nc, tc, g, with_collectives, rg):
    """Fused SwiGLU + down projection, one token half at a time.
    h never leaves SBUF; the 1/rms factor of rmsnorm2 (with the fp8 weight
    descale folded in) is applied to the gate/up psums.  ReduceScatter
    chunks (by output-row group) fire as the second half completes them."""
    with (
        tc.tile_pool(name="p45w", bufs=1) as pw,
        tc.tile_pool(name="p45big", bufs=1) as pb45,
        tc.tile_pool(name="p45sbuf", bufs=2) as ps,
    ):
        pp = g["psum"]
        hh8 = pb45.tile([128, KI, SH], FP8, name="hh8")
        hl8 = pb45.tile([128, KI, SH], FP8, name="hl8")
        # rstd for all tokens from the gathered sum-of-squares rows; the
        # Sqrt scale/bias are pre-multiplied by 2^12 so the reciprocal
        # comes out as rstd * 2^-6 (the fp8 weight descale rides along)
        sums8 = pw.tile([8, TPC], F32, name="sums8")
        nc.sync.dma_start(sums8[:], g["sums_out"][:, :])
        std8 = pw.tile([8, TPC], F32, name="std8")
        nc.scalar.activation(std8[:], sums8[:], AF.Sqrt,
                             bias=g["epsb8"][:], scale=SW * SW / H)
        rstd8 = pw.tile([8, TPC], F32, name="rstd8")
        nc.vector.reciprocal(rstd8[:], std8[:])
        nc.sync.dma_start(g["rstd_dram"][0, :, :], rstd8[:])
        rstd_row = g["rstd1_sb"]   # rmsnorm1's row tile, dead after phase 1
        nc.sync.dma_start(rstd_row[:], g["rstd_dram"][0:1, :, :])

        xnha, xnla = g["xnha"], g["xnla"]
        xnhb, xnlb = g["xnhb"], g["xnlb"]
        for T in range(2):
            tsl = slice(T * SH, (T + 1) * SH)
            rbh = ps.tile([128, SH], F32, name="rbh", tag="rbh", bufs=1)
            nc.gpsimd.partition_broadcast(rbh[:], rstd_row[:, tsl])
            # gate/up: 256-token column blocks (one gathered source core per
            # block, so the cp-major xn layout keeps 3D matmul APs)
            for m in range(MB_GU):
                if T == 0 and m == 0:
                    gbh, ubh = g["gb0h"], g["ub0h"]   # preloaded during o_proj
                    gbl, ubl = g["gb0l"], g["ub0l"]
                else:
                    gbh = ps.tile([128, KH, 128], FP8, name="gbh", tag="wgu", bufs=6)
                    nc.sync.dma_start(gbh[:], g["wguh"][:, m, :, :])
                    gbl = ps.tile([128, KH, 128], FP8, name="gbl", tag="wgu", bufs=6)
                    nc.sync.dma_start(gbl[:], g["wgul"][:, m, :, :])
                    ubh = ps.tile([128, KH, 128], FP8, name="ubh", tag="wgu", bufs=6)
                    nc.sync.dma_start(ubh[:], g["wguh"][:, MB_GU + m, :, :])
                    ubl = ps.tile([128, KH, 128], FP8, name="ubl", tag="wgu", bufs=6)
                    nc.sync.dma_start(ubl[:], g["wgul"][:, MB_GU + m, :, :])
                for tb in range(4):
                    lcols = slice(tb * 256, (tb + 1) * 256)
                    g_ps = pp.tile([128, 256], F32, name="g_ps", tag="s_ps", bufs=4)
                    _dr3(nc, g_ps, gbh, gbl, xnha, xnla, 26, first=True,
                         last=False, xmid=tb)
                    _dr3(nc, g_ps, gbh, gbl, xnhb, xnlb, 6, wk0=26,
                         first=False, last=True, xmid=tb)
                    u_ps = pp.tile([128, 256], F32, name="u_ps", tag="s_ps", bufs=4)
                    _dr3(nc, u_ps, ubh, ubl, xnha, xnla, 26, first=True,
                         last=False, xmid=tb)
                    _dr3(nc, u_ps, ubh, ubl, xnhb, xnlb, 6, wk0=26,
                         first=False, last=True, xmid=tb)
                    gsc = ps.tile([128, 256], F32R, name="gsc", tag="gsc", bufs=2)
                    nc.vector.tensor_mul(gsc[:], g_ps[:], rbh[:, lcols])
                    usc = ps.tile([128, 256], F32R, name="usc", tag="usc", bufs=2)
                    nc.vector.tensor_mul(usc[:], u_ps[:], rbh[:, lcols])
                    sg = ps.tile([128, 256], F32R, name="sg", tag="sg", bufs=2)
                    nc.scalar.activation(sg[:], gsc[:], AF.Silu)
                    h32 = ps.tile([128, 256], F32R, name="h32", tag="h32", bufs=2)
                    nc.vector.tensor_mul(h32[:], sg[:], usc[:])
                    nc.vector.tensor_copy(hh8[:, m, lcols], h32[:])
                    nc.vector.tensor_sub(hl8[:, m, lcols], h32[:], hh8[:, m, lcols])
            if T == 0:
                # second token half of the gathered activations: issue now so
                # they stream during the first half's down projection (the
                # WAR on xn resolved when the last gate/up matmul above ran)
                for cch, (off, sz) in enumerate(AG_CHUNKS):
                    xh_t = xnha if off < 26 else xnhb
                    xl_t = xnla if off < 26 else xnlb
                    loff = off if off < 26 else off - 26
                    for cp in range(4):
                        nc.gpsimd.dma_start(
                            xh_t[:, cp, loff:loff + sz, :],
                            g[f"ag2_out{cch}h"][(4 + cp) * 128:(5 + cp) * 128, :, :],
                        )
                        nc.gpsimd.dma_start(
                            xl_t[:, cp, loff:loff + sz, :],
                            g[f"ag2_out{cch}l"][(4 + cp) * 128:(5 + cp) * 128, :, :],
                        )
            if T == 1:
                # residual output, deferred into the last down stretch
                for cch, (off, sz) in enumerate(AG_CHUNKS):
                    nc.sync.dma_start(g["res_out"][:, off:off + sz, :],
                                      g[f"x2b{cch}"][:])
            # down
            for r in range(8):
                for mi in range(KH // 8):
                    m = r * (KH // 8) + mi
                    dbh = ps.tile([128, KI, 128], FP8, name="dbh", tag="db", bufs=4)
                    nc.sync.dma_start(dbh[:], g["wdnh"][:, m, :, :])
                    dbl = ps.tile([128, KI, 128], FP8, name="dbl", tag="db", bufs=4)
                    nc.sync.dma_start(dbl[:], g["wdnl"][:, m, :, :])
                    ot = ps.tile([128, SH], BF16, name="ot", tag="ot", bufs=2)
                    for tb in range(2):
                        lcols = slice(tb * 512, (tb + 1) * 512)
                        d_ps = pp.tile([128, 512], F32, name="d_ps", tag="mm", bufs=2)
                        # KI=14 k-tiles -> 7 DoubleRow pairs
                        _dr3(nc, d_ps, dbh, dbl, hh8, hl8, KI, ncols=lcols,
                             first=True, last=True)
                        # descale on the psum copy (ACT, off the DVE)
                        nc.scalar.activation(ot[:, lcols], d_ps[:], AF.Copy,
                                             scale=DSW)
                    nc.sync.dma_start(g[f"rs_in{r}"][mi * 128:(mi + 1) * 128, tsl], ot[:])
                if T == 1:
                    if with_collectives:
                        nc.gpsimd.collective_compute(
                            "ReduceScatter", mybir.AluOpType.add, replica_groups=rg,
                            ins=[g[f"rs_in{r}"].opt()], outs=[g[f"rs_out{r}"].opt()],
                        )
                    else:
                        nc.sync.dma_start(g[f"rs_out{r}"][:, :],
                                          g[f"rs_in{r}"][0:H // NC // 8, :])
                    # upconvert the bf16 shard to the fp32 output
                    ob = ps.tile([64, S], BF16, name="ob", tag="ob", bufs=1)
                    nc.gpsimd.dma_start(ob[:], g[f"rs_out{r}"][:, :])
                    for hf in range(4):
                        hsl2 = slice(hf * 512, (hf + 1) * 512)
                        of = ps.tile([64, 512], F32, name="of", tag="of", bufs=2)
                        nc.vector.tensor_copy(of[:], ob[:, hsl2])
                        nc.gpsimd.dma_start(g["out_down"][r * 64:(r + 1) * 64, hsl2], of[:])


def build_program(with_collectives=True, stop_after=99):
    nc = bacc.Bacc("TRN2", target_bir_lowering=False, debug=False, num_devices=NC)

    g = {}
    g["hTbh"] = nc.dram_tensor("hTbh", [128, KH, S], FP8, kind="ExternalInput")
    g["hTbl"] = nc.dram_tensor("hTbl", [128, KH, S], FP8, kind="ExternalInput")
    g["hT_slice"] = nc.dram_tensor("hT_slice", [128, KH, TPC], BF16, kind="ExternalInput")
    g["wqkvh"] = nc.dram_tensor("wqkvh", [128, QH + 2, KH, 128], FP8, kind="ExternalInput")
    g["wqkvl"] = nc.dram_tensor("wqkvl", [128, QH + 2, KH, 128], FP8, kind="ExternalInput")
    g["woh"] = nc.dram_tensor("woh", [128, KH, QH, 128], FP8, kind="ExternalInput")
    g["wol"] = nc.dram_tensor("wol", [128, KH, QH, 128], FP8, kind="ExternalInput")
    g["wguh"] = nc.dram_tensor("wguh", [128, 2 * MB_GU, KH, 128], FP8, kind="ExternalInput")
    g["wgul"] = nc.dram_tensor("wgul", [128, 2 * MB_GU, KH, 128], FP8, kind="ExternalInput")
    g["wdnh"] = nc.dram_tensor("wdnh", [128, KH, KI, 128], FP8, kind="ExternalInput")
    g["wdnl"] = nc.dram_tensor("wdnl", [128, KH, KI, 128], FP8, kind="ExternalInput")
    g["cosT"] = nc.dram_tensor("cosT", [128, S], BF16, kind="ExternalInput")
    g["sinT"] = nc.dram_tensor("sinT", [128, S], BF16, kind="ExternalInput")
    g["rstd1"] = nc.dram_tensor("rstd1", [1, S], F32, kind="ExternalInput")
    g["masks"] = nc.dram_tensor("masks", [128, 4, 512], BF16, kind="ExternalInput")

    g["res_out"] = nc.dram_tensor("res_out", [128, KH, TPC], BF16, kind="ExternalOutput")
    g["out_down"] = nc.dram_tensor("out_down", [H // NC, S], F32, kind="ExternalOutput")

    rg = [list(range(NC))]

    with tile.TileContext(nc) as tc:
        with (
            tc.tile_pool(name="consts", bufs=1) as consts,
            tc.tile_pool(name="dram", bufs=1, space="DRAM") as dram,
        ):
            shr = {"addr_space": "Shared"} if with_collectives else {}
            for cch, (off, sz) in enumerate(AG_CHUNKS):
                g[f"rsp_in{cch}"] = dram.tile([NC * 128, sz, TPC], BF16,
                                              name=f"rsp_in{cch}")
                g[f"rsp_out{cch}"] = dram.tile([128, sz, TPC], BF16,
                                               name=f"rsp_out{cch}")
                for sfx in ("h", "l"):
                    g[f"ag2_in{cch}{sfx}"] = dram.tile([128, sz, TPC], FP8,
                                                       name=f"ag2_in{cch}{sfx}")
                    g[f"ag2_out{cch}{sfx}"] = dram.tile([NC * 128, sz, TPC], FP8,
                                                        name=f"ag2_out{cch}{sfx}", **shr)
            g["sums_in"] = dram.tile([1, TPC], F32, name="sums_in")
            g["sums_out"] = dram.tile([NC, TPC], F32, name="sums_out", addr_space="Shared")
            g["rstd_dram"] = dram.tile([1, NC, TPC], F32, name="rstd_dram")
            for r in range(8):
                g[f"rs_in{r}"] = dram.tile([H // 8, S], BF16, name=f"rs_in{r}")
                g[f"rs_out{r}"] = dram.tile([H // NC // 8, S], BF16, name=f"rs_out{r}")

            ones32 = consts.tile([128, 1], F32, name="ones32")
            nc.gpsimd.memset(ones32[:], 1.0)
            g["ones"] = consts.tile([128, 1], F32R, name="ones")
            nc.vector.tensor_copy(g["ones"][:], ones32[:])
            g["ones_b"] = consts.tile([128, 1], BF16, name="ones_b")
            nc.vector.tensor_copy(g["ones_b"][:], ones32[:])
            ident32 = consts.tile([128, 128], F32, name="ident32")
            make_identity(nc, ident32[:])
            g["ident"] = consts.tile([128, 128], BF16, name="ident")
            nc.vector.tensor_copy(g["ident"][:], ident32[:])
            g["epsb8"] = consts.tile([8, 1], F32, name="epsb8")
            nc.gpsimd.memset(g["epsb8"][:], EPS * SW * SW)
            g["rstd1_sb"] = consts.tile([1, S], F32, name="rstd1_sb")

            # reserved early: hsl preloads during the DMA-quiet qkv window;
            # the fp8 hi/lo context tiles are written during attention and
            # contracted by the row-parallel o_proj in phase 3
            psum_cm = tc.tile_pool(name="psum", bufs=1, space="PSUM")
            g["psum"] = psum_cm.__enter__()
            wop_cm = tc.tile_pool(name="wopre", bufs=1)
            wop = wop_cm.__enter__()
            g["hsl_sb"] = wop.tile([128, KH, TPC], BF16, name="hsl_sb")
            g["ctxh_sb"] = wop.tile([128, QH, S], FP8, name="ctxh_sb")
            g["ctxl_sb"] = wop.tile([128, QH, S], FP8, name="ctxl_sb")

            with tc.tile_pool(name="attn", bufs=1) as attn:
                g["mask_sb"] = attn.tile([128, 4, 512], BF16, name="mask_sb")
                g["qT_sb"] = attn.tile([128, QH, S], BF16, name="qT_sb")          # 2 MB
                g["kT_sb"] = attn.tile([128, S], BF16, name="kT_sb")              # 0.5 MB
                g["v_tok"] = attn.tile([128, S // 128, 128], BF16, name="v_tok")  # 0.5 MB

                with tc.tile_pool(name="p2sbuf", bufs=2) as p2s:
                    _phase12_qkv_attn(nc, tc, g, p2s, with_collectives, rg)

            if stop_after >= 3:
                with tc.tile_pool(name="mlpbig", bufs=1) as pb:
                    g["xnha"] = pb.tile([128, 4, 26, TPC], FP8, name="xnha")
                    g["xnla"] = pb.tile([128, 4, 26, TPC], FP8, name="xnla")
                    g["xnhb"] = pb.tile([128, 4, 6, TPC], FP8, name="xnhb")
                    g["xnlb"] = pb.tile([128, 4, 6, TPC], FP8, name="xnlb")
                    g["gb0h"] = pb.tile([128, KH, 128], FP8, name="gb0h")
                    g["gb0l"] = pb.tile([128, KH, 128], FP8, name="gb0l")
                    g["ub0h"] = pb.tile([128, KH, 128], FP8, name="ub0h")
                    g["ub0l"] = pb.tile([128, KH, 128], FP8, name="ub0l")
                    _phase3_oproj(nc, tc, g, with_collectives, rg)
                    if stop_after >= 4:
                        _phase45_mlp(nc, tc, g, with_collectives, rg)
            wop_cm.__exit__(None, None, None)
            psum_cm.__exit__(None, None, None)

    nc.finalize()
    return nc


_cached_nc = None


def _get_nc():
    global _cached_nc
    if _cached_nc is None:
        _cached_nc = build_program(with_collectives=True)
    return _cached_nc


def _split8(a):
    """fp8 e4m3 hi/lo split of a float32 array."""
    hi = a.astype(E4NP)
    lo = (a - hi.astype(np.float32)).astype(E4NP)
    return hi, lo


def _host_prep(positions, hidden_states, w_qkv, w_o, w_gate_up, w_down, ln1_w, ln2_w):
    f32 = np.float32
    bf16 = ml_dtypes.bfloat16
    hidden = np.asarray(hidden_states, dtype=f32)[0]          # [S, H]
    hT = np.ascontiguousarray(hidden.T)                        # [H, S]
    hTb_np = np.ascontiguousarray(
        hT.reshape(KH, 128, S).transpose(1, 0, 2))             # [128, KH, S] f32
    hTbh_np, hTbl_np = _split8(hTb_np)
    pos = np.asarray(positions).astype(f32)[0]                 # [S]

    half = HD // 2
    inv_freq = (1.0 / (f32(THETA) ** (np.arange(0, half, dtype=f32) / f32(half)))).astype(f32)
    ang = pos[:, None] * inv_freq[None, :]                     # [S, 64] fp32
    # rmsnorm1 only depends on the input: precompute 1/rms per token and
    # fold it (and the fp8 weight descale 2^-6) into the rope tables (both
    # commute through the QKV matmul)
    rstd1_np = (1.0 / np.sqrt((hidden.astype(np.float64) ** 2).mean(axis=1) + EPS)
                ).astype(f32)                                  # [S]
    cos_half = (np.cos(ang).astype(f32) * (rstd1_np * f32(DSW))[:, None]).T  # [64, S]
    sin_half = (np.sin(ang).astype(f32) * (rstd1_np * f32(DSW))[:, None]).T
    cosT_np = np.ascontiguousarray(np.concatenate([cos_half, cos_half], axis=0)).astype(bf16)  # [128, S]
    sinT_np = np.ascontiguousarray(np.concatenate([sin_half, sin_half], axis=0)).astype(bf16)
    rstd1_row = np.ascontiguousarray((rstd1_np * f32(DSW)).reshape(1, S))

    w_qkv_f = np.asarray(w_qkv, dtype=f32) * np.asarray(ln1_w, dtype=f32)[:, None] * f32(SW)
    w_gu_f = np.asarray(w_gate_up, dtype=f32) * np.asarray(ln2_w, dtype=f32)[:, None] * f32(SW)
    w_o_f32 = np.asarray(w_o, dtype=f32) * f32(SW)             # [NQ*HD, H]
    w_dn_f = np.asarray(w_down, dtype=f32) * f32(SW)

    kk = np.arange(128)[:, None, None]
    jj = np.arange(4)[None, :, None]
    qq = np.arange(512)[None, None, :]
    masks_np = np.ascontiguousarray((qq >= kk + 128 * jj).astype(bf16))  # [128, 4, 512]

    in_maps = []
    for c in range(NC):
        # o_proj row slice for this core's 4 heads -> [128, m, head_kt, 128]
        wo_c = np.ascontiguousarray(
            w_o_f32[c * QH * HD:(c + 1) * QH * HD, :]
            .reshape(QH, 128, KH, 128).transpose(1, 2, 0, 3))
        woh_c, wol_c = _split8(wo_c)
        q_cols = w_qkv_f[:, c * QH * HD:(c + 1) * QH * HD]
        k_col = w_qkv_f[:, NQ * HD + c * HD: NQ * HD + (c + 1) * HD]
        v_col = w_qkv_f[:, (NQ + NKV) * HD + c * HD: (NQ + NKV) * HD + (c + 1) * HD]
        wqkv_c = np.concatenate([q_cols, k_col, v_col], axis=1)
        wqkv_c = np.ascontiguousarray(
            wqkv_c.reshape(KH, 128, QH + 2, 128).transpose(1, 2, 0, 3))
        wqkvh_c, wqkvl_c = _split8(wqkv_c)
        wgu_c = np.concatenate(
            [w_gu_f[:, c * IPC:(c + 1) * IPC],
             w_gu_f[:, I + c * IPC: I + (c + 1) * IPC]], axis=1)
        wgu_c = np.ascontiguousarray(
            wgu_c.reshape(KH, 128, 2 * MB_GU, 128).transpose(1, 2, 0, 3))
        wguh_c, wgul_c = _split8(wgu_c)
        wdn_c = np.ascontiguousarray(
            w_dn_f[c * IPC:(c + 1) * IPC, :].reshape(KI, 128, KH, 128).transpose(1, 2, 0, 3))
        wdnh_c, wdnl_c = _split8(wdn_c)
        hT_slice_c = np.ascontiguousarray(
            hT[:, c * TPC:(c + 1) * TPC].reshape(KH, 128, TPC).transpose(1, 0, 2)
        ).astype(bf16)
        in_maps.append({
            "hTbh": hTbh_np,
            "hTbl": hTbl_np,
            "hT_slice": hT_slice_c,
            "wqkvh": wqkvh_c,
            "wqkvl": wqkvl_c,
            "woh": woh_c,
            "wol": wol_c,
            "wguh": wguh_c,
            "wgul": wgul_c,
            "wdnh": wdnh_c,
            "wdnl": wdnl_c,
            "cosT": cosT_np,
            "sinT": sinT_np,
            "rstd1": rstd1_row,
            "masks": masks_np,
        })
    return in_maps


def kernel(**inputs):
    in_maps = _host_prep(**inputs)
    nc = _get_nc()
    res = run_bass_kernel_spmd(nc, in_maps, core_ids=list(range(NC)))
    results = res.results

    outT = np.empty((H, S), np.float32)
    for c in range(NC):
        od = results[c]["out_down"]           # [512, S]: chunk r rows -> global 512r+64c
        for r in range(8):
            outT[512 * r + 64 * c: 512 * r + 64 * (c + 1)] = od[64 * r:64 * (r + 1)]
    resT = np.concatenate(
        [np.asarray(results[c]["res_out"]).astype(np.float32)
         .transpose(1, 0, 2).reshape(H, TPC)
         for c in range(NC)], axis=1)          # [H, S]
    out = np.ascontiguousarray(outT.T).reshape(1, S, H).astype(np.float32)
    residual = np.ascontiguousarray(resT.T).reshape(1, S, H).astype(np.float32)
    return out, residual
